# revision 1
# baseline (speedup 1.0000x reference)
"""Trainium2 Bass kernel for nn_CombinedRepeatCausalLinear (PE version).

Math: out[r, t] = sum_{s<=t} x[r, s] * (w0[s]*dv0^(t-s) + w1[t]*dv1^(t-s)) + bias[t]

Chunked linear-attention formulation (chunk L=126 along S):
  - Diagonal blocks D_c[s_l, t_l] (upper-triangular, [128,128] with 2 extra
    "reduction" columns producing decay-weighted chunk sums S0_c, S1_c in
    psum rows 126/127).
  - Cross-chunk contribution is rank-2 per source chunk:
      out[t] += sum_{c'<c(t)} dv0^(t-e_c')*S1_c'[r] + w1[t]*dv1^(t-e_c')*S0_c'[r]
    implemented as a second matmul per chunk against a host-built matrix.

All matmuls are K=128 (host matrices zero-padded) so the PE array stays
fully active and the HAM clock-gate holds the warm 2.4 GHz state; fp32
throughout (HI/LO 2-pass, exact fp32 products). The output is computed
transposed (t on partitions) so the host-built matrices are always the
stationary operand; the host ships x pre-transposed per shard and
transposes the gathered result back.

Data-parallel across 8 NeuronCores on the fused B*E axis.
"""

import sys

if "/opt/trn_rl_repo" not in sys.path:
    sys.path.insert(0, "/opt/trn_rl_repo")

import numpy as np

import concourse.mybir as mybir
from concourse import bacc
from concourse.bass_utils import run_bass_kernel_spmd
from concourse.mybir import AluOpType
from concourse.tile import TileContext

_P = 128
_B, _E, _S = 4, 2048, 2048
_NCORES = 8
_R = (_B * _E) // _NCORES  # 1024 rows (r) per core
_L = 126  # chunk length along S
_NCH = (_S + _L - 1) // _L  # 17 chunks (last has 32)
_HALF = 512  # r per matmul (one PSUM bank, fp32)
_NH = _R // _HALF  # 2 halves

_F32 = mybir.dt.float32


def _chunk_len(c):
    return min(_L, _S - c * _L)


def _build_host_mats(w0, w1, dv0, dv1, bias, with_bias):
    """Build D [128, NCH*128] and M [128, NCH*128] in float64, cast f32."""
    w0 = w0.astype(np.float64)
    w1 = w1.astype(np.float64)
    D = np.zeros((_NCH * _P, _P), dtype=np.float64)
    for c in range(_NCH):
        Lc = _chunk_len(c)
        base = c * _L
        sl = np.arange(Lc)
        tl = np.arange(Lc)
        diff = tl[None, :] - sl[:, None]
        mask = diff >= 0
        blk = np.where(
            mask,
            w0[base + sl][:, None] * (dv0 ** np.maximum(diff, 0))
            + w1[base + tl][None, :] * (dv1 ** np.maximum(diff, 0)),
            0.0,
        )
        Db = D[c * _P : (c + 1) * _P]
        Db[:Lc, :Lc] = blk
        # reduction columns: col 126 -> S0_c (dv1-weighted sum),
        #                    col 127 -> S1_c (w0*dv0-weighted sum)
        Db[:Lc, 126] = dv1 ** (Lc - 1 - sl)
        Db[:Lc, 127] = w0[base + sl] * dv0 ** (Lc - 1 - sl)

    off = 1 if with_bias else 0
    # M padded to 128 contraction rows (rows >= off+2*NCH are zero)
    M = np.zeros((_P, _NCH * _P), dtype=np.float64)
    for c in range(_NCH):
        Lc = _chunk_len(c)
        t = c * _L + np.arange(Lc)
        if with_bias:
            M[0, c * _P : c * _P + Lc] = bias.astype(np.float64)[t]
        for cp in range(c):
            e_cp = cp * _L + _chunk_len(cp) - 1
            M[off + 2 * cp, c * _P : c * _P + Lc] = w1[t] * (dv1 ** (t - e_cp))
            M[off + 2 * cp + 1, c * _P : c * _P + Lc] = dv0 ** (t - e_cp)
    return D.astype(np.float32), M.astype(np.float32)


def _build(with_bias):
    off = 1 if with_bias else 0
    nc = bacc.Bacc(
        "TRN2",
        target_bir_lowering=False,
        debug=False,
        enable_asserts=False,
        num_devices=_NCORES,
    )
    xt = nc.dram_tensor("xt", [_S, _R], _F32, kind="ExternalInput").ap()
    Dd = nc.dram_tensor("Dd", [_NCH * _P, _P], _F32, kind="ExternalInput").ap()
    Md = nc.dram_tensor("Md", [_P, _NCH * _P], _F32, kind="ExternalInput").ap()
    outT = nc.dram_tensor("outT", [_S, _R], _F32, kind="ExternalOutput").ap()

    with TileContext(nc) as tc:
        with (
            tc.tile_pool(name="consts", bufs=1) as cpool,
            tc.tile_pool(name="xin", bufs=8) as xpool,
            tc.tile_pool(name="dg", bufs=1) as dgpool,
            tc.tile_pool(name="ot", bufs=4) as otpool,
            tc.tile_pool(name="pd", bufs=5, space="PSUM") as pdpool,
            tc.tile_pool(name="po", bufs=3, space="PSUM") as popool,
        ):
            sall = cpool.tile([_P, _R], _F32)
            nc.gpsimd.memset(sall[:], 0.0)
            if with_bias:
                nc.gpsimd.memset(sall[0:1, :], 1.0)
            # dedicated last-chunk x tile, zero-filled up front so the
            # memset is off the phase-1 critical path (K=128 contraction
            # reads the zero tail rows)
            xlast = cpool.tile([_P, _R], _F32)
            nc.gpsimd.memset(xlast[:], 0.0)

            # ---- Phase 1: diagonal blocks + chunk reductions ----
            Dt = None
            Mt = None
            dg_tiles = []
            for c in range(_NCH):
                Lc = _chunk_len(c)
                rows = min(_P, _S - c * _L)  # 128, except last chunk: 32
                if rows < _P:
                    xtile = xlast
                else:
                    xtile = xpool.tile([_P, _R], _F32, tag="x", name="x")
                nc.sync.dma_start(xtile[:rows, :], xt[c * _L : c * _L + rows, :])
                dtile = xpool.tile([_P, _P], _F32, tag="d", name="d")
                nc.scalar.dma_start(dtile[:], Dd[c * _P : (c + 1) * _P, :])
                if c == 8:
                    # M is only needed for phase 2; load it mid-phase
                    Mt = cpool.tile([_P, _NCH * _P], _F32)
                    nc.scalar.dma_start(Mt[:], Md[:])
                dg = dgpool.tile([_P, _R], _F32, tag=f"dg{c}", name="dg")
                for h in range(_NH):
                    pd = pdpool.tile([_P, _HALF], _F32, tag="pd", name="pd")
                    nc.tensor.matmul(
                        pd[:],
                        dtile[:],
                        xtile[:, h * _HALF : (h + 1) * _HALF],
                        start=True,
                        stop=True,
                    )
                    nc.vector.tensor_copy(dg[:, h * _HALF : (h + 1) * _HALF], pd[:])
                    # move the chunk-sum rows into Sall partitions (2c, 2c+1)
                    nc.gpsimd.dma_start(
                        sall[off + 2 * c : off + 2 * c + 2, h * _HALF : (h + 1) * _HALF],
                        dg[126:128, h * _HALF : (h + 1) * _HALF],
                    )
                dg_tiles.append(dg)

            # ---- Phase 2: cross-chunk offsets + combine + store ----
            for c in range(_NCH):
                Lc = _chunk_len(c)
                dg = dg_tiles[c]
                if c == 0 and not with_bias:
                    nc.sync.dma_start(outT[0:_L, :], dg[:_L, :])
                    continue
                ot = otpool.tile([_P, _R], _F32, tag="ot", name="ot")
                for h in range(_NH):
                    po = popool.tile([_P, _HALF], _F32, tag="po", name="po")
                    nc.tensor.matmul(
                        po[:],
                        Mt[:, c * _P : (c + 1) * _P],
                        sall[:, h * _HALF : (h + 1) * _HALF],
                        start=True,
                        stop=True,
                    )
                    nc.vector.tensor_tensor(
                        ot[:, h * _HALF : (h + 1) * _HALF],
                        dg[:, h * _HALF : (h + 1) * _HALF],
                        po[:],
                        AluOpType.add,
                    )
                eng = nc.sync if c % 2 == 0 else nc.scalar
                eng.dma_start(outT[c * _L : c * _L + Lc, :], ot[:Lc, :])
    nc.compile()
    return nc


def _run(x, weight, bias, decay_value, trace=False):
    x = np.asarray(x, dtype=np.float32)
    w = np.asarray(weight, dtype=np.float32)
    b = np.asarray(bias, dtype=np.float32)
    dv = np.asarray(decay_value, dtype=np.float32)
    dv0 = float(np.clip(dv[0, 0], 0.9, 1.0))
    dv1 = float(np.clip(dv[1, 0], 0.9, 1.0))
    with_bias = bool(np.any(b))

    D, M = _build_host_mats(w[0], w[1], dv0, dv1, b, with_bias)
    nc = _build(with_bias)

    xf = x.reshape(_B * _E, _S)
    xT = np.ascontiguousarray(xf.T)  # [S, B*E]
    in_maps = []
    for c in range(_NCORES):
        in_maps.append(
            {
                "xt": np.ascontiguousarray(xT[:, c * _R : (c + 1) * _R]),
                "Dd": D,
                "Md": M,
            }
        )

    res = run_bass_kernel_spmd(nc, in_maps, core_ids=list(range(_NCORES)), trace=trace)
    outT = np.concatenate(
        [res.results[c]["outT"] for c in range(_NCORES)], axis=1
    )  # [S, B*E]
    full = np.ascontiguousarray(outT.T).reshape(_B, _E, _S)
    return full, res


def kernel(x, weight, bias, decay_value):
    full, _ = _run(x, weight, bias, decay_value, trace=False)
    return full



# revision 4
# speedup vs baseline: 1.3244x; 1.3244x over previous
"""Trainium2 Bass kernel for nn_CombinedRepeatCausalLinear (fp16 v2).

Math: out[r, t] = sum_{s<=t} x[r, s] * (w0[s]*dv0^(t-s) + w1[t]*dv1^(t-s)) + bias[t]

Chunked formulation (chunk L=126 along S), all matmuls fp16 (1 cycle/row on
the PE vs 4 for fp32), psum accumulation fp32:

  Pass A  (chunk sums):   S0(c) = sum_{s in c} x[s,r]*dv1^(b_{c+1}-s)
                          S1(c) = sum_{s in c} x[s,r]*w0[s]*dv0^(b_{c+1}-s)
          one K=126 matmul per chunk accumulating into a [128,512] psum
          (rows 2c / 2c+1), per r-half.
  Prefix  (tiny matmul):  P0(c) = sum_{c'<c} dv1^(b_c-b_{c'+1}) S0(c'), etc.
          via a host-built [34,34] (padded 128) prefix matrix.
  Scatter: P0(c),P1(c) rows are DMA'd (SBUF->SBUF) into rows 126/127 of the
          per-chunk x tile.
  Pass C  (main): ONE K=128 matmul per chunk-half: rows 0..125 are the
          upper-triangular diagonal decay block, rows 126/127 multiply the
          prefix values (D[126,t]=w1[t]*dv1^tl, D[127,t]=dv0^tl), so the
          cross-chunk rank-2 term rides in the same matmul.

Output is fp16 (psum->sbuf copy downcasts, split across DVE and Act
engines), halving the output DMA; bias is added on the host after gather.
The host ships x pre-transposed fp16 per shard ([S, R] layout, t on
partitions in the kernel output) and transposes the gathered result back.

Data-parallel across 8 NeuronCores on the fused B*E axis.
"""

import sys

if "/opt/trn_rl_repo" not in sys.path:
    sys.path.insert(0, "/opt/trn_rl_repo")

import numpy as np

import concourse.mybir as mybir
from concourse import bacc
from concourse.bass_utils import run_bass_kernel_spmd
from concourse.tile import TileContext

_P = 128
_B, _E, _S = 4, 2048, 2048
_NCORES = 8
_R = (_B * _E) // _NCORES  # 1024 rows (r) per core
_L = 126  # chunk length along S
_NCH = (_S + _L - 1) // _L  # 17 chunks (last has 32)
_H = 512  # r per matmul (one PSUM bank, fp32)

_F32 = mybir.dt.float32
_F16 = mybir.dt.float16


def _chunk_len(c):
    return min(_L, _S - c * _L)


def _build_host_mats(w0, w1, dv0, dv1):
    """DM [128, NCH*128], SA [128, NCH*128], PR [128, 128] in f64, cast f16."""
    w0 = w0.astype(np.float64)
    w1 = w1.astype(np.float64)
    DM = np.zeros((_P, _NCH * _P), dtype=np.float64)
    SA = np.zeros((_P, _NCH * _P), dtype=np.float64)
    PR = np.zeros((_P, _P), dtype=np.float64)
    for c in range(_NCH):
        Lc = _chunk_len(c)
        b = c * _L
        sl = np.arange(Lc)
        tl = np.arange(Lc)
        diff = tl[None, :] - sl[:, None]
        mask = diff >= 0
        e = np.where(mask, diff, 0)
        blk = np.where(
            mask,
            w0[b + sl][:, None] * (dv0**e) + w1[b + tl][None, :] * (dv1**e),
            0.0,
        )
        DM[:Lc, c * _P : c * _P + Lc] = blk
        DM[126, c * _P : c * _P + Lc] = w1[b + tl] * dv1**tl
        DM[127, c * _P : c * _P + Lc] = dv0**tl
        if c < _NCH - 1:
            # chunk sums relative to the next chunk start b_{c+1} = b + Lc
            SA[sl, c * _P + 2 * c] = dv1 ** (Lc - sl)
            SA[sl, c * _P + 2 * c + 1] = w0[b + sl] * dv0 ** (Lc - sl)
    for c in range(1, _NCH):
        for cp in range(c):
            g = _L * (c - cp - 1)  # b_c - b_{cp+1}
            PR[2 * cp, 2 * c] = dv1**g
            PR[2 * cp + 1, 2 * c + 1] = dv0**g
    return (
        DM.astype(np.float16),
        SA.astype(np.float16),
        PR.astype(np.float16),
    )


def _build():
    nc = bacc.Bacc(
        "TRN2",
        target_bir_lowering=False,
        debug=False,
        enable_asserts=False,
        num_devices=_NCORES,
    )
    xt = nc.dram_tensor("xt", [_S, _R], _F16, kind="ExternalInput").ap()
    DMd = nc.dram_tensor("DMd", [_P, _NCH * _P], _F16, kind="ExternalInput").ap()
    SAd = nc.dram_tensor("SAd", [_P, _NCH * _P], _F16, kind="ExternalInput").ap()
    PRd = nc.dram_tensor("PRd", [_P, _P], _F16, kind="ExternalInput").ap()
    outT = nc.dram_tensor("outT", [_S, _R], _F16, kind="ExternalOutput").ap()

    with TileContext(nc) as tc:
        with (
            tc.tile_pool(name="consts", bufs=1) as cpool,
            tc.tile_pool(name="ot", bufs=3) as opool,
            tc.tile_pool(name="pw", bufs=1, space="PSUM") as pwpool,
            tc.tile_pool(name="pap", bufs=1, space="PSUM") as papool,
            tc.tile_pool(name="pc", bufs=3, space="PSUM") as pcpool,
        ):
            DM16 = cpool.tile([_P, _NCH * _P], _F16)
            SA16 = cpool.tile([_P, _NCH * _P], _F16)
            PR16 = cpool.tile([_P, _P], _F16)
            xbig = cpool.tile([_P, _NCH * _R], _F16)
            sall = cpool.tile([_P, _R], _F16)
            P16 = cpool.tile([_P, _R], _F16)
            zz = cpool.tile([_P, _H], _F16)

            nc.gpsimd.memset(zz[:], 0.0)
            # last chunk has 32 real rows; zero the tail so the K=128 pass-C
            # contraction never reads garbage (NaN*0 = NaN). Memset APs must
            # start 32-aligned with <=32 partitions.
            for p0 in range(32, 128, 32):
                nc.gpsimd.memset(xbig[p0 : p0 + 32, 16 * _R : 17 * _R], 0.0)

            nc.scalar.dma_start(PR16[:], PRd[:])
            nc.scalar.dma_start(SA16[:], SAd[:])
            for c in range(_NCH):
                Lc = _chunk_len(c)
                nc.sync.dma_start(
                    xbig[0:Lc, c * _R : c * _R + _R], xt[c * _L : c * _L + Lc, :]
                )
            nc.scalar.dma_start(DM16[:], DMd[:])

            # PE p-state warm-up: ~3us of continuous dummy matmuls so the
            # real passes run at the full 2.4 GHz clock
            for _ in range(8):
                pw = pwpool.tile([_P, _H], _F32, tag="warm", name="warm")
                nc.tensor.matmul(pw[:], PR16[:], zz[:], start=True, stop=True)

            # ---- Pass A: per-chunk decay-weighted sums ----
            psA = [
                papool.tile([_P, _H], _F32, tag=f"psA{h}", name="psA")
                for h in range(2)
            ]
            for c in range(_NCH - 1):
                for h in range(2):
                    nc.tensor.matmul(
                        psA[h][:],
                        SA16[0:126, c * _P : (c + 1) * _P],
                        xbig[0:126, c * _R + h * _H : c * _R + (h + 1) * _H],
                        start=(c == 0),
                        stop=(c == _NCH - 2),
                    )
            nc.vector.tensor_copy(sall[:, 0:_H], psA[0][:])
            nc.scalar.copy(sall[:, _H : 2 * _H], psA[1][:])

            # ---- Prefix: P(c) = sum over earlier chunks, one tiny matmul ----
            psP = [
                papool.tile([_P, _H], _F32, tag=f"psP{h}", name="psP")
                for h in range(2)
            ]
            for h in range(2):
                nc.tensor.matmul(
                    psP[h][:],
                    PR16[:],
                    sall[:, h * _H : (h + 1) * _H],
                    start=True,
                    stop=True,
                )
            nc.vector.tensor_copy(P16[:, 0:_H], psP[0][:])
            nc.scalar.copy(P16[:, _H : 2 * _H], psP[1][:])

            # ---- Scatter prefix rows into rows 126/127 of each x block ----
            sceng = [nc.gpsimd, nc.sync, nc.scalar]
            for c in range(_NCH):
                sceng[c % 3].dma_start(
                    xbig[126:128, c * _R : (c + 1) * _R], P16[2 * c : 2 * c + 2, :]
                )

            # ---- Pass C: one K=128 matmul per chunk-half ----
            for c in range(_NCH):
                Lc = _chunk_len(c)
                o16 = opool.tile([_P, _R], _F16, tag="o", name="o")
                for h in range(2):
                    pc_t = pcpool.tile([_P, _H], _F32, tag="pc", name="pc")
                    nc.tensor.matmul(
                        pc_t[:],
                        DM16[:, c * _P : (c + 1) * _P],
                        xbig[:, c * _R + h * _H : c * _R + (h + 1) * _H],
                        start=True,
                        stop=True,
                    )
                    if (c + h) % 2 == 0:
                        nc.vector.tensor_copy(
                            o16[0:Lc, h * _H : (h + 1) * _H], pc_t[0:Lc, :]
                        )
                    else:
                        nc.scalar.copy(
                            o16[0:Lc, h * _H : (h + 1) * _H], pc_t[0:Lc, :]
                        )
                oeng = nc.sync if c % 2 == 0 else nc.gpsimd
                oeng.dma_start(outT[c * _L : c * _L + Lc, :], o16[0:Lc, :])
    nc.compile()
    return nc


def _run(x, weight, bias, decay_value, trace=False):
    x = np.asarray(x, dtype=np.float32)
    w = np.asarray(weight, dtype=np.float32)
    b = np.asarray(bias, dtype=np.float32)
    dv = np.asarray(decay_value, dtype=np.float32)
    dv0 = float(np.clip(dv[0, 0], 0.9, 1.0))
    dv1 = float(np.clip(dv[1, 0], 0.9, 1.0))

    DM, SA, PR = _build_host_mats(w[0], w[1], dv0, dv1)
    nc = _build()

    xT = np.ascontiguousarray(x.reshape(_B * _E, _S).astype(np.float16).T)
    in_maps = []
    for c in range(_NCORES):
        in_maps.append(
            {
                "xt": np.ascontiguousarray(xT[:, c * _R : (c + 1) * _R]),
                "DMd": DM,
                "SAd": SA,
                "PRd": PR,
            }
        )

    res = run_bass_kernel_spmd(nc, in_maps, core_ids=list(range(_NCORES)), trace=trace)
    outT = np.concatenate(
        [res.results[c]["outT"] for c in range(_NCORES)], axis=1
    )  # [S, B*E] fp16
    out = outT.T.astype(np.float32) + b[None, :]
    return np.ascontiguousarray(out).reshape(_B, _E, _S), res


def kernel(x, weight, bias, decay_value):
    full, _ = _run(x, weight, bias, decay_value, trace=False)
    return full
